# revision 35
# baseline (speedup 1.0000x reference)
"""Trainium2 Bass kernel for nn_DemandRouter (retrieval kNN).

Reference computation (per batch b):
    Q = x @ Wq.T + bq          [T, 32]
    K = x @ Wk.T + bk          [T, 32]
    sim = Q @ K.T / sqrt(32)   [T, T]
    idx = top_k(sim, 4)        [T, 4]
    out[t] = mean(x[idx[t]])   [T, D]

Sharding: 8 cores = 4 batches x 2 T-halves (data parallel over B, then
split the query rows T; every core projects keys for all T of its
batch). Each core receives x[b] ROLLED so its own 1024 query rows come
first — sim columns, top-k indices and the gather table all live in the
same rolled coordinate system, so the program is identical across cores
(SPMD) with no on-device offsets.

Measured bottleneck (this container, slope method): the kernel is NOT
byte-bound. Ablations show (a) the 8 MiB xrt load + projection hide
completely under phase C/D (removing them changes nothing), (b) each
indirect-gather DMA op costs ~1.06 us marginal — the single SWDGE
queue is a serial descriptor-gen/drain pipe — so the 32 gather ops
per iteration cost ~34 us on top of (c) a ~48 us serial per-tile
sim -> max -> max_index chain (psim bufs=1; all 8 PSUM banks are
committed: 4 projection + 4 sim). Design choices:

  - The host passes x[b] transposed (xrt) so the d-contraction runs
    directly off DMA-loaded tiles — no on-device transposes.
  - Gather table AND output are fp16 (halves those bytes); the top-k
    decision path (projection + sim) stays exact fp32 — float32r
    (~13-bit) flips top-k near-ties, measured 0.025 rel err. fp16
    gather/output adds only ~3.8e-4 rel err vs the 2e-2 gate.
  - The host pre-scales the gather table by 0.25 (exact power of two),
    so the 4-neighbor mean is just 3 DVE adds.
  - The 1/sqrt(32) sim scale is dropped (argmax-invariant).
  - Top-4 comes from the DVE max/max_index top-8 unit reading the sim
    PSUM tile directly (no PSUM->SBUF copy of sim).
  - Persistent tile pools + 2-tile software skew pipeline repeats
    (loads of r+1 under the gather tail of r); loads on the SP HWDGE
    ring, stores on ACT, gathers on SWDGE.
  - ~4us of dummy matmuls ramp the PE p-state under the first DMA.

Per-core pipeline:
  A. stream xrt d-row tiles [128, 2048]; accumulate Wqk^T.T @ xrt into
     4 PSUM banks -> [Q;K]^T [64, 2048] (contract d in 8 chunks).
  B. PSUM -> SBUF with per-partition bias add (ScalarE).
  C. per 128-row t-tile: sim = Q^T.T @ K^T into a 4-bank PSUM tile
     [128, 2048]; DVE max/max_index -> top-8 values+indices.
  D. 4 single-index indirect-DMA gathers of fp16 0.25x rows, 3 DVE
     adds; store the 128x1024 fp16 output tile (host converts to f32).

Dead ends (measured, do not retry blindly):
  - Multi-index indirect gathers (offset ap with 2 or 4 columns):
    runtime INTERNAL crash. One offset per partition is the only
    supported shape, so 4 ops/tile is the indirect_dma_start floor.
  - Routing indirect DMAs to qPoolDynamic{1..3} (num_swdge_queues=4):
    NRT_EXEC_UNIT_UNRECOVERABLE device crash.
  - Intra-pair AllGather of K^T halves (KERNEL_PAIR=1): the ncfw
    collective costs ~80 us per iteration.
  - Deeper gather skew (KERNEL_SKEW=4): no change — the gather cost
    is op-serialization, not DVE waiting.
Next step if revisited: batch gathers through dma_gather
(InstDMAGatherAnt, supports num_idxs=512+ per op) — needs int16
[16, n/16]-wrapped indices via the digit-swap permutation
pi(p) = 16*(p%8) + p//8 and a host-side row un-permute of the output.
A partial attempt lives behind KERNEL_DGATH=1 (default OFF — NOT yet
correct): the u32->int16 cast-DMA marshal zeroes index values (SWDGE
cast treats data as float), a strided bitcast source AP crashes
INTERNAL, and the uint16-max_index + contiguous bitcast version runs
but yields NaN (placement bug undiagnosed). A KERNEL_DGATH_DEBUG=1
build dumping ix/ixt/ixw/ga to extra outputs (checker: dbg_dgath.py)
itself crashes INTERNAL — pare it to one plain 2D contiguous dump per
build when resuming. Also check whether the idxs tile needs a
128-partition pitch (the interp views idxs as (128, n/16) and reads
partitions [:16]).
"""

import os

import numpy as np

import concourse.bass as bass
import concourse.mybir as mybir
import concourse.tile as tile
from concourse import bacc
from concourse.bass import ts
from concourse.bass_utils import run_bass_kernel_spmd

B, T, D = 4, 2048, 1024
KQ = 32          # query/key projection width
KTOP = 4
P = 128
N_CORES = 8
TQ = T // 2      # query rows handled per core
ND = D // P      # 8 contraction chunks of 128
NG = 4           # t column-groups of full T
GT = T // NG     # 512 t per group
NGH = 2          # t column-groups of own half
NT = TQ // P     # 8 query row-tiles per core

f32 = mybir.dt.float32
f32r = mybir.dt.float32r
f16 = mybir.dt.float16
u32 = mybir.dt.uint32
IDENT = mybir.ActivationFunctionType.Identity

# experiment flags (read at module build time)
USE_F32R = os.environ.get("KERNEL_F32R", "0") == "1"
USE_CCE = os.environ.get("KERNEL_CCE", "0") == "1"
# fp16 gather table + fp16 output: halves gather (16->8 MiB) and store
# (4->2 MiB) HBM traffic; top-k sim path stays exact f32. Host converts
# the output back to f32. fp16 mantissa (2^-11) adds ~5e-4 rel err,
# nowhere near the 2e-2 gate.
USE_F16G = os.environ.get("KERNEL_F16G", "1") == "1"
USE_PAIR = os.environ.get("KERNEL_PAIR", "0") == "1"
ABLATE = os.environ.get("KERNEL_ABLATE", "")
# fused 2-index gathers + stores on the ACT HWDGE ring + deeper pools
USE_V2CD = os.environ.get("KERNEL_V2CD", "1") == "1"
USE_GIDX2 = os.environ.get("KERNEL_GIDX2", "0") == "1"
# single fused 4-index gather per query tile (one SWDGE op, 512 descs).
# CRASHES the runtime (INTERNAL error on HW) — multi-column offset APs
# are not supported by the indirect-DMA descriptor path. Keep off.
USE_GIDX4 = os.environ.get("KERNEL_GIDX4", "0") == "1"
# persistent pools + software pipelining across repeats (phase A of
# repeat r+1 overlaps the gather/store tail of repeat r)
USE_PIPE = os.environ.get("KERNEL_PIPE", "1") == "1"
# spread indirect gathers across N SWDGE queues (1-4). Each queue is a
# serial desc-gen/drain pipe (~1.06us per gather op measured); multiple
# rings would let gather ops overlap — but routing plain InstDMACopy to
# qPoolDynamic{1..3} crashes the device (NRT_EXEC_UNIT_UNRECOVERABLE);
# only the dma_gather/scatter Ant family honors queue_num. Keep 1.
N_SWQ = int(os.environ.get("KERNEL_SWQ", "1"))
# batched dma_gather: 1 marshal DMA + 1 InstDMAGatherAnt per tile
# (2 SWDGE ops) instead of 4 indirect ops. Gather position i=j*128+pi(q)
# with pi(q)=16*(q%8)+q//8 so the u32->int16 wrapped-index marshal is one
# AP-rearranged cast DMA; the host un-permutes output rows by pi.
USE_DGATH = os.environ.get("KERNEL_DGATH", "0") == "1"
# dump dgath pipeline stages (ix/ixt/ixw/ga of tile 0) to DRAM outputs
DGATH_DEBUG = os.environ.get("KERNEL_DGATH_DEBUG", "0") == "1"
# pi as a gather map for the host-side row unpermute
PI = np.array([16 * (q % 8) + q // 8 for q in range(P)], dtype=np.int64)

# float32r is *rounded* fp32 (reduced precision) — measured 0.025 rel err
# on this problem, so it stays off; exact fp32 everywhere.
MM_DT = f32r if USE_F32R else f32
# gather-table / output dtype (never feeds the top-k decision)
G_DT = f16 if USE_F16G else f32
G_NP = np.float16 if USE_F16G else np.float32

PAIR_GROUPS = [[0, 1], [2, 3], [4, 5], [6, 7]]

_NC = None


def _emit_warmup(tc, nc):
    from contextlib import ExitStack

    # ~4us of dummy matmuls so the PE p-state ramps to 2.4 GHz while the
    # first input DMA is in flight. Pools scoped so the PSUM bank frees
    # before phase C needs all 8.
    with ExitStack() as wctx:
        wu = wctx.enter_context(tc.tile_pool(name="wu", bufs=1))
        wups = wctx.enter_context(tc.tile_pool(name="wups", bufs=1, space="PSUM"))
        wsb = wu.tile([P, P], f32)
        nc.gpsimd.memset(wsb[:], 1.0)
        wps = wups.tile([P, P], f32)
        for _ in range(10):
            nc.tensor.matmul(wps[:], lhsT=wsb[:], rhs=wsb[:], start=True, stop=True)


def _emit_topk_gather(tc, nc, pcd, qt, kt, xg, out):
    """Phases C+D: sim, top-k, gather, mean, store."""
    psim = pcd.enter_context(tc.tile_pool(name="psim", bufs=2, space="PSUM"))
    gpool = pcd.enter_context(tc.tile_pool(name="gpool", bufs=4 if USE_V2CD else 2))
    mpool = pcd.enter_context(tc.tile_pool(name="mpool", bufs=3))
    opool = pcd.enter_context(tc.tile_pool(name="opool", bufs=3 if USE_V2CD else 2))

    for i in range(NT):
        simp = psim.tile([P, T], f32, tag="sim", name=f"sim{i}")
        for c in range(NG):
            nc.tensor.matmul(
                simp[:, ts(c, GT)],
                lhsT=qt[:, ts(i, P)],
                rhs=kt[:, ts(c, GT)],
                start=True,
                stop=True,
            )
        mx = mpool.tile([P, 8], f32, tag="mx", name=f"mx{i}")
        ix = mpool.tile([P, 8], u32, tag="ix", name=f"ix{i}")
        nc.vector.max(out=mx[:], in_=simp[:])
        nc.vector.max_index(out=ix[:], in_max=mx[:], in_values=simp[:])

        if ABLATE == "nogather":
            g = [
                gpool.tile([P, D], G_DT, tag=f"g{k}", name=f"g{k}_{i}")
                for k in range(2)
            ]
            nc.gpsimd.memset(g[0][:], 0.5)
            nc.gpsimd.memset(g[1][:], 0.25)
            s01 = opool.tile([P, D], G_DT, tag="s01", name=f"s01_{i}")
            nc.vector.tensor_add(s01[:], g[0][:], g[1][:])
        elif USE_GIDX4:
            # one fused 4-index gather: ga[p, j, :] = xg[ix[p, j]] for
            # j=0..3 — a single SWDGE op with 512 descriptors, then a
            # 2-level DVE add tree for the mean.
            ga = gpool.tile([P, KTOP, D], G_DT, tag="ga", name=f"ga_{i}")
            nc.gpsimd.indirect_dma_start(
                out=ga[:],
                out_offset=None,
                in_=xg[:, :],
                in_offset=bass.IndirectOffsetOnAxis(ap=ix[:, 0:KTOP], axis=0),
            )
            s01 = opool.tile([P, D], G_DT, tag="s01", name=f"s01_{i}")
            s23 = opool.tile([P, D], G_DT, tag="s23", name=f"s23_{i}")
            nc.vector.tensor_add(s01[:], ga[:, 0, :], ga[:, 1, :])
            nc.vector.tensor_add(s23[:], ga[:, 2, :], ga[:, 3, :])
            nc.vector.tensor_add(s01[:], s01[:], s23[:])
        elif USE_V2CD and USE_GIDX2:
            # two fused 2-index gathers: ga[p, j, :] = xg[ix[p, j]] then
            # += xg[ix[p, j+2]] via cce add; one DVE add folds j=0,1.
            ga = gpool.tile([P, 2, D], G_DT, tag="ga", name=f"ga_{i}")
            nc.gpsimd.indirect_dma_start(
                out=ga[:],
                out_offset=None,
                in_=xg[:, :],
                in_offset=bass.IndirectOffsetOnAxis(ap=ix[:, 0:2], axis=0),
            )
            nc.gpsimd.indirect_dma_start(
                out=ga[:],
                out_offset=None,
                in_=xg[:, :],
                in_offset=bass.IndirectOffsetOnAxis(ap=ix[:, 2:4], axis=0),
                compute_op=mybir.AluOpType.add,
            )
            s01 = opool.tile([P, D], G_DT, tag="s01", name=f"s01_{i}")
            nc.vector.tensor_add(s01[:], ga[:, 0, :], ga[:, 1, :])
        elif USE_CCE:
            g = [
                gpool.tile([P, D], G_DT, tag=f"g{k}", name=f"g{k}_{i}")
                for k in range(2)
            ]
            for k in range(KTOP):
                nc.gpsimd.indirect_dma_start(
                    out=g[k % 2][:],
                    out_offset=None,
                    in_=xg[:, :],
                    in_offset=bass.IndirectOffsetOnAxis(ap=ix[:, k : k + 1], axis=0),
                    compute_op=(
                        mybir.AluOpType.add if k >= 2 else mybir.AluOpType.bypass
                    ),
                )
            s01 = opool.tile([P, D], G_DT, tag="s01", name=f"s01_{i}")
            nc.vector.tensor_add(s01[:], g[0][:], g[1][:])
        else:
            # 4 plain gathers + 3 DVE adds. CCE-add gathers re-read the
            # destination tile through the SBUF AXI ports, so with the
            # fp16 table plain-gather has lower port traffic than CCE.
            g = [
                gpool.tile([P, D], G_DT, tag=f"g{k}", name=f"g{k}_{i}")
                for k in range(KTOP)
            ]
            for k in range(KTOP):
                nc.gpsimd.indirect_dma_start(
                    out=g[k][:],
                    out_offset=None,
                    in_=xg[:, :],
                    in_offset=bass.IndirectOffsetOnAxis(ap=ix[:, k : k + 1], axis=0),
                )
            s01 = opool.tile([P, D], G_DT, tag="s01", name=f"s01_{i}")
            s23 = opool.tile([P, D], G_DT, tag="s23", name=f"s23_{i}")
            nc.vector.tensor_add(s01[:], g[0][:], g[1][:])
            nc.vector.tensor_add(s23[:], g[2][:], g[3][:])
            nc.vector.tensor_add(s01[:], s01[:], s23[:])
        # xg rows are pre-scaled by 0.25 on the host (exact power-of-two
        # scale), so s01 already is the 4-neighbor mean. Stores alternate
        # between the ACT and SP HWDGE rings (SP is idle during C/D).
        if USE_V2CD:
            seng = nc.scalar if i % 2 == 0 else nc.sync
            seng.dma_start(out[ts(i, P), :], s01[:])
        else:
            nc.sync.dma_start(out[ts(i, P), :], s01[:])


def _emit_pair(tc, nc, xg, xth, wqkt, bqk, out, warmup):
    """Pair-sharing variant: project own T-half only, AllGather K^T.

    Everything is in GLOBAL coordinates: sim columns are global t, the
    gather table xg is the unrolled x[b], and the output rows are the
    core's own global query rows.
    """
    from contextlib import ExitStack

    with ExitStack() as ctx:
        if warmup:
            _emit_warmup(tc, nc)
        cpool = ctx.enter_context(tc.tile_pool(name="consts", bufs=1))
        wq_sb = cpool.tile([P, ND, 2 * KQ], MM_DT)  # [128, 8, 64]; d = dd*128+p
        nc.sync.dma_start(wq_sb[:], wqkt.rearrange("(n p) k -> p n k", p=P))
        bqk_sb = cpool.tile([2 * KQ, 1], f32)
        nc.sync.dma_start(bqk_sb[:], bqk[:])
        qt = cpool.tile([KQ, TQ], f32)  # Q^T (own half) with bias
        kt = cpool.tile([KQ, NGH, TQ], f32)  # K^T (full T) with bias

        dpool = ctx.enter_context(tc.tile_pool(name="ccdram", bufs=1, space="DRAM"))
        cc_in = dpool.tile([KQ, TQ], f32)
        cc_out = dpool.tile([2 * KQ, TQ], f32)

        # ---- phase A: load own xth half + project ----
        with ExitStack() as pa:
            xt_pool = pa.enter_context(tc.tile_pool(name="xt", bufs=3))
            pqkt = pa.enter_context(tc.tile_pool(name="pqkt", bufs=1, space="PSUM"))
            qk_ps = [
                pqkt.tile([2 * KQ, GT], f32, tag=f"qk{c}", name=f"qk_ps{c}")
                for c in range(NGH)
            ]
            kth = cpool.tile([KQ, TQ], f32)  # own biased K^T half
            if ABLATE == "noproj":
                nc.vector.memset(qt[:], 0.001)
                nc.vector.memset(kth[:], 0.002)
            for dd in range(ND if ABLATE != "noproj" else 0):
                xt = xt_pool.tile([P, TQ], MM_DT, tag="xt", name=f"xt{dd}")
                nc.sync.dma_start(xt[:], xth[ts(dd, P), :])
                for c in range(NGH):
                    nc.tensor.matmul(
                        qk_ps[c][:],
                        lhsT=wq_sb[:, dd, :],
                        rhs=xt[:, ts(c, GT)],
                        start=(dd == 0),
                        stop=(dd == ND - 1),
                    )

            # ---- phase B: PSUM -> SBUF with bias ----
            for c in range(NGH if ABLATE != "noproj" else 0):
                nc.scalar.activation(
                    qt[:, ts(c, GT)], qk_ps[c][0:KQ, :], IDENT, bias=bqk_sb[0:KQ, :]
                )
                nc.scalar.activation(
                    kth[:, ts(c, GT)],
                    qk_ps[c][KQ : 2 * KQ, :],
                    IDENT,
                    bias=bqk_sb[KQ : 2 * KQ, :],
                )
        nc.sync.dma_start(cc_in[:], kth[:])
        nc.gpsimd.collective_compute(
            "AllGather",
            mybir.AluOpType.bypass,
            replica_groups=PAIR_GROUPS,
            ins=[cc_in[:]],
            outs=[cc_out[:]],
        )
        # cc_out rows [0:32] = pair rank 0 (global t 0..1023), rows
        # [32:64] = pair rank 1 — global column order for both cores.
        nc.sync.dma_start(kt[:], cc_out.rearrange("(h k) s -> k h s", k=KQ))

        with ExitStack() as pcd:
            _emit_topk_gather(
                tc, nc, pcd, qt, kt.rearrange("k h s -> k (h s)"), xg, out
            )


def _emit_solo(tc, nc, xg, xrt, wqkt, bqk, out, warmup):
    """Original variant: every core projects all T keys itself (rolled
    coordinates: the core's queries are rows [0:1024) of the rolled x)."""
    from contextlib import ExitStack

    with ExitStack() as ctx:
        if warmup:
            _emit_warmup(tc, nc)
        cpool = ctx.enter_context(tc.tile_pool(name="consts", bufs=1))
        wq_sb = cpool.tile([P, ND, 2 * KQ], MM_DT)
        nc.sync.dma_start(wq_sb[:], wqkt.rearrange("(n p) k -> p n k", p=P))
        bqk_sb = cpool.tile([2 * KQ, 1], f32)
        nc.sync.dma_start(bqk_sb[:], bqk[:])
        qt = cpool.tile([KQ, T], f32)
        kt = cpool.tile([KQ, T], f32)

        with ExitStack() as pa:
            xt_pool = pa.enter_context(tc.tile_pool(name="xt", bufs=3))
            pqkt = pa.enter_context(tc.tile_pool(name="pqkt", bufs=1, space="PSUM"))
            qk_ps = [
                pqkt.tile([2 * KQ, GT], f32, tag=f"qk{c}", name=f"qk_ps{c}")
                for c in range(NG)
            ]
            if ABLATE == "noproj":
                nc.vector.memset(qt[:], 0.001)
                nc.vector.memset(kt[:], 0.002)
            for dd in range(ND if ABLATE != "noproj" else 0):
                xt = xt_pool.tile([P, T], MM_DT, tag="xt", name=f"xt{dd}")
                # alternate load issue across both HWDGE rings (SP/ACT)
                eng = nc.sync if (dd % 2 == 0 or not USE_V2CD) else nc.scalar
                eng.dma_start(xt[:], xrt[ts(dd, P), :])
                for c in range(NG):
                    nc.tensor.matmul(
                        qk_ps[c][:],
                        lhsT=wq_sb[:, dd, :],
                        rhs=xt[:, ts(c, GT)],
                        start=(dd == 0),
                        stop=(dd == ND - 1),
                    )
            for c in range(NG if ABLATE != "noproj" else 0):
                nc.scalar.activation(
                    qt[:, ts(c, GT)], qk_ps[c][0:KQ, :], IDENT, bias=bqk_sb[0:KQ, :]
                )
                nc.scalar.activation(
                    kt[:, ts(c, GT)],
                    qk_ps[c][KQ : 2 * KQ, :],
                    IDENT,
                    bias=bqk_sb[KQ : 2 * KQ, :],
                )

        with ExitStack() as pcd:
            _emit_topk_gather(tc, nc, pcd, qt, kt, xg, out)


# tiles of pipeline skew between a tile's gather issue and its DVE mean
# (so the DVE queue never stalls waiting on an in-flight gather DMA)
SKEW = int(os.environ.get("KERNEL_SKEW", "2"))


def _make_pools(tc, ctx):
    """Persistent pools shared by every repeat so consecutive repeats
    overlap: repeat r+1's xrt load + projection runs under repeat r's
    gather/store tail. PSUM budget: pqkt 4 banks + psim 4 banks = 8."""
    pl = {}
    pl["qk"] = ctx.enter_context(tc.tile_pool(name="qk", bufs=2))
    pl["xt"] = ctx.enter_context(tc.tile_pool(name="xt", bufs=3))
    pl["pqkt"] = ctx.enter_context(tc.tile_pool(name="pqkt", bufs=1, space="PSUM"))
    pl["psim"] = ctx.enter_context(tc.tile_pool(name="psim", bufs=1, space="PSUM"))
    pl["gp"] = ctx.enter_context(tc.tile_pool(name="gp", bufs=SKEW + 2))
    pl["mp"] = ctx.enter_context(tc.tile_pool(name="mp", bufs=3))
    pl["op"] = ctx.enter_context(tc.tile_pool(name="op", bufs=3))
    return pl


def _emit_iter(tc, nc, pl, xg, xrt, out, r):
    """One repeat of the solo kernel on the persistent pools."""
    wq_sb, bqk_sb = pl["wq_sb"], pl["bqk_sb"]

    # ---- phase A: stream xrt, project QK^T into PSUM ----
    qt = pl["qk"].tile([KQ, T], f32, tag="qt", name=f"qt_{r}")
    kt = pl["qk"].tile([KQ, T], f32, tag="kt", name=f"kt_{r}")
    qk_ps = [
        pl["pqkt"].tile([2 * KQ, GT], f32, tag=f"qk{c}", name=f"qk_ps{c}_{r}")
        for c in range(NG)
    ]
    if ABLATE == "noproj":
        nc.vector.memset(qt[:], 0.001)
        nc.vector.memset(kt[:], 0.002)
    for dd in range(ND if ABLATE != "noproj" else 0):
        xt = pl["xt"].tile([P, T], MM_DT, tag="xt", name=f"xt{dd}_{r}")
        # loads stay on the SP HWDGE ring; stores own the ACT ring, so a
        # store waiting on a late gather never queues ahead of r+1 loads
        nc.sync.dma_start(xt[:], xrt[ts(dd, P), :])
        for c in range(NG):
            nc.tensor.matmul(
                qk_ps[c][:],
                lhsT=wq_sb[:, dd, :],
                rhs=xt[:, ts(c, GT)],
                start=(dd == 0),
                stop=(dd == ND - 1),
            )
    # ---- phase B: PSUM -> SBUF with per-partition bias ----
    for c in range(NG if ABLATE != "noproj" else 0):
        nc.scalar.activation(
            qt[:, ts(c, GT)], qk_ps[c][0:KQ, :], IDENT, bias=bqk_sb[0:KQ, :]
        )
        nc.scalar.activation(
            kt[:, ts(c, GT)],
            qk_ps[c][KQ : 2 * KQ, :],
            IDENT,
            bias=bqk_sb[KQ : 2 * KQ, :],
        )

    # ---- phases C+D: sim, top-k, fused gather, mean, store ----
    pend = []

    def flush():
        ga, idx = pend.pop(0)
        s01 = pl["op"].tile([P, D], G_DT, tag="s01", name=f"s01_{idx}_{r}")
        s23 = pl["op"].tile([P, D], G_DT, tag="s23", name=f"s23_{idx}_{r}")
        nc.vector.tensor_add(s01[:], ga[:, 0, :], ga[:, 1, :])
        nc.vector.tensor_add(s23[:], ga[:, 2, :], ga[:, 3, :])
        nc.vector.tensor_add(s01[:], s01[:], s23[:])
        nc.scalar.dma_start(out[ts(idx, P), :], s01[:])

    for i in range(NT):
        simp = pl["psim"].tile([P, T], f32, tag="sim", name=f"sim{i}_{r}")
        for c in range(NG):
            nc.tensor.matmul(
                simp[:, ts(c, GT)],
                lhsT=qt[:, ts(i, P)],
                rhs=kt[:, ts(c, GT)],
                start=True,
                stop=True,
            )
        mx = pl["mp"].tile([P, 8], f32, tag="mx", name=f"mx{i}_{r}")
        # uint16 indices for the dma_gather path (its idxs must be 2-byte;
        # a dtype-cast DMA mangles integers, so emit 2-byte at the source)
        ix = pl["mp"].tile(
            [P, 8], mybir.dt.uint16 if USE_DGATH else u32, tag="ix",
            name=f"ix{i}_{r}",
        )
        nc.vector.max(out=mx[:], in_=simp[:])
        nc.vector.max_index(out=ix[:], in_max=mx[:], in_values=simp[:])
        ga = pl["gp"].tile([P, KTOP, D], G_DT, tag="ga", name=f"ga_{i}_{r}")
        if USE_DGATH and not ABLATE:
            # marshal ix[q, j] (u32, one query per partition) into the
            # wrapped int16 index layout dma_gather wants: position
            # i = j*128 + pi(q), pi(q) = 16*(q%8) + q//8, lives at
            # [i % 16, i // 16] = [q//8, j*8 + q%8].
            # Stage 1: cast-DMA in source-natural order — dest (c, g, j)
            # zips 1:1 with source (q = 8c+g, j).
            ixt = pl["mp"].tile([16, 8, KTOP], mybir.dt.int16, tag="ixt",
                                name=f"ixt_{i}_{r}")
            # plain contiguous 2-byte move (uint16 ix bitcast to int16)
            nc.gpsimd.dma_start(ixt[:], ix.bitcast(mybir.dt.int16)[:, 0:KTOP])
            # Stage 2: tiny DVE permute (g, j) -> (j, g) within partitions
            ixw = pl["mp"].tile([16, KTOP, 8], mybir.dt.int16, tag="ixw",
                                name=f"ixw_{i}_{r}")
            nc.vector.tensor_copy(ixw[:], ixt.rearrange("c g j -> c j g"))
            # one gather op for all 512 rows: ga[pi(q), j, :] = xg[ix[q, j]]
            nc.gpsimd.dma_gather(
                out_ap=ga[:],
                in_ap=xg[:, :],
                idxs_ap=ixw.rearrange("c j g -> c (j g)"),
                num_idxs=P * KTOP,
                num_idxs_reg=P * KTOP,
                elem_size=D,
            )
            if DGATH_DEBUG and r == 0 and i == 0:
                dbg = pl["dbg"]
                nc.sync.dma_start(dbg["ix"], ix[:])
                nc.sync.dma_start(dbg["ixt"], ixt.rearrange("c g j -> c (g j)"))
                nc.sync.dma_start(dbg["ixw"], ixw.rearrange("c j g -> c (j g)"))
                nc.scalar.dma_start(dbg["ga"], ga[:])
        elif ABLATE == "nogather":
            nc.gpsimd.memset(ga[:, 0:1, :], 0.25)
            nc.gpsimd.memset(ga[:, 1:2, :], 0.125)
            nc.gpsimd.memset(ga[:, 2:3, :], 0.0625)
            nc.gpsimd.memset(ga[:, 3:4, :], 0.03125)
        elif USE_GIDX4:
            # one fused 4-index gather: ga[p, j, :] = xg[ix[p, j]]
            # (a single SWDGE op, 512 descriptors of one row each)
            nc.gpsimd.indirect_dma_start(
                out=ga[:],
                out_offset=None,
                in_=xg[:, :],
                in_offset=bass.IndirectOffsetOnAxis(ap=ix[:, 0:KTOP], axis=0),
            )
        elif USE_GIDX2:
            # two fused 2-index gathers (halves the SWDGE op count vs
            # one op per neighbor)
            for h in range(2):
                nc.gpsimd.indirect_dma_start(
                    out=ga[:, 2 * h : 2 * h + 2, :],
                    out_offset=None,
                    in_=xg[:, :],
                    in_offset=bass.IndirectOffsetOnAxis(
                        ap=ix[:, 2 * h : 2 * h + 2], axis=0
                    ),
                )
        else:
            # ABLATE=gather2/gather1: timing probes with fewer gather ops
            # per tile (results wrong; calibrates per-op SWDGE cost)
            ng = {"gather2": 2, "gather1": 1}.get(ABLATE, KTOP)
            for k in range(ng):
                inst = nc.gpsimd.indirect_dma_start(
                    out=ga[:, k, :],
                    out_offset=None,
                    in_=xg[:, :],
                    in_offset=bass.IndirectOffsetOnAxis(ap=ix[:, k : k + 1], axis=0),
                )
                if N_SWQ > 1:
                    q = (i * KTOP + k) % N_SWQ
                    inst.ins.queue = f"qPoolDynamic{q or ''}"
        pend.append((ga, i))
        if len(pend) > SKEW:
            flush()
    while pend:
        flush()


def _build_module():
    repeat = int(os.environ.get("KERNEL_REPEAT", "1"))
    nc = bacc.Bacc(
        "TRN2",
        target_bir_lowering=False,
        debug=False,
        num_devices=N_CORES,
        num_swdge_queues=N_SWQ,
    )
    if USE_PAIR:
        xg = nc.dram_tensor("xg", [T, D], G_DT, kind="ExternalInput").ap()
        xth = nc.dram_tensor("xth", [D, TQ], MM_DT, kind="ExternalInput").ap()
        wqkt = nc.dram_tensor("wqkt", [D, 2 * KQ], MM_DT, kind="ExternalInput").ap()
        bqk = nc.dram_tensor("bqk", [2 * KQ, 1], f32, kind="ExternalInput").ap()
        out = nc.dram_tensor("out", [TQ, D], G_DT, kind="ExternalOutput").ap()
        with tile.TileContext(nc) as tc:
            for r in range(repeat):
                _emit_pair(tc, nc, xg, xth, wqkt, bqk, out, warmup=(r == 0))
    else:
        xg = nc.dram_tensor("xr", [T, D], G_DT, kind="ExternalInput").ap()
        xrt = nc.dram_tensor("xrt", [D, T], MM_DT, kind="ExternalInput").ap()
        wqkt = nc.dram_tensor("wqkt", [D, 2 * KQ], MM_DT, kind="ExternalInput").ap()
        bqk = nc.dram_tensor("bqk", [2 * KQ, 1], f32, kind="ExternalInput").ap()
        out = nc.dram_tensor("out", [TQ, D], G_DT, kind="ExternalOutput").ap()
        from contextlib import ExitStack

        with tile.TileContext(nc) as tc:
            if USE_PIPE:
                with ExitStack() as ctx:
                    _emit_warmup(tc, nc)
                    cpool = ctx.enter_context(tc.tile_pool(name="consts", bufs=1))
                    pl = _make_pools(tc, ctx)
                    pl["wq_sb"] = cpool.tile([P, ND, 2 * KQ], MM_DT, name="wq_sb")
                    nc.sync.dma_start(
                        pl["wq_sb"][:], wqkt.rearrange("(n p) k -> p n k", p=P)
                    )
                    pl["bqk_sb"] = cpool.tile([2 * KQ, 1], f32, name="bqk_sb")
                    nc.sync.dma_start(pl["bqk_sb"][:], bqk[:])
                    if DGATH_DEBUG:
                        pl["dbg"] = {
                            "ix": nc.dram_tensor(
                                "dbg_ix", [P, 8], mybir.dt.uint16,
                                kind="ExternalOutput").ap(),
                            "ixt": nc.dram_tensor(
                                "dbg_ixt", [16, 32], mybir.dt.int16,
                                kind="ExternalOutput").ap(),
                            "ixw": nc.dram_tensor(
                                "dbg_ixw", [16, 32], mybir.dt.int16,
                                kind="ExternalOutput").ap(),
                            "ga": nc.dram_tensor(
                                "dbg_ga", [P, KTOP, D], G_DT,
                                kind="ExternalOutput").ap(),
                        }
                    for r in range(repeat):
                        _emit_iter(tc, nc, pl, xg, xrt, out, r)
            else:
                for r in range(repeat):
                    _emit_solo(tc, nc, xg, xrt, wqkt, bqk, out, warmup=(r == 0))
    nc.compile()
    return nc


def _get_nc():
    global _NC
    if _NC is None:
        _NC = _build_module()
    return _NC


def _make_in_maps(x, Wq, bq, Wk, bk):
    x = np.ascontiguousarray(np.asarray(x, dtype=np.float32))
    wqkt = np.ascontiguousarray(
        np.concatenate(
            [np.asarray(Wq, np.float32).T, np.asarray(Wk, np.float32).T], axis=1
        )
    )
    bqk = np.concatenate(
        [np.asarray(bq, np.float32), np.asarray(bk, np.float32)]
    )[:, None]
    bqk = np.ascontiguousarray(bqk)
    in_maps = []
    # gather tables: pre-scaled by 0.25 (exact power of two), in G_NP
    # (fp16 rounding never feeds the top-k decision, only gathered rows)
    xq = (x * np.float32(0.25)).astype(G_NP)
    for c in range(N_CORES):
        b, h = divmod(c, 2)
        off = h * TQ
        xb = x[b]
        if USE_PAIR:
            in_maps.append(
                {
                    "xg": np.ascontiguousarray(xq[b]),
                    "xth": np.ascontiguousarray(xb[off : off + TQ].T),
                    "wqkt": wqkt,
                    "bqk": bqk,
                }
            )
        else:
            xrc = (
                np.concatenate([xq[b][off:], xq[b][:off]], axis=0)
                if off
                else xq[b]
            )
            in_maps.append(
                {
                    "xr": np.ascontiguousarray(xrc),
                    "xrt": np.ascontiguousarray(xb.T) if off == 0 else
                           np.ascontiguousarray(
                               np.concatenate([xb[off:], xb[:off]], axis=0).T),
                    "wqkt": wqkt,
                    "bqk": bqk,
                }
            )
    return in_maps


def run(x, Wq, bq, Wk, bk, trace=False):
    """Run on 8 cores; returns (full_output, BassKernelResults)."""
    in_maps = _make_in_maps(x, Wq, bq, Wk, bk)
    nc = _get_nc()
    res = run_bass_kernel_spmd(nc, in_maps, list(range(N_CORES)), trace=trace)
    outf = np.empty((B, T, D), np.float32)
    for c in range(N_CORES):
        b, h = divmod(c, 2)
        blk = res.results[c]["out"].astype(np.float32)
        if USE_DGATH:
            # device row pi(q) of each 128-row tile holds query q
            blk = blk.reshape(NT, P, D)[:, PI, :].reshape(TQ, D)
        outf[b, h * TQ : (h + 1) * TQ] = blk
    return outf, res


def kernel(x, Wq, bq, Wk, bk):
    outf, _ = run(x, Wq, bq, Wk, bk, trace=False)
    return outf

